# revision 15
# baseline (speedup 1.0000x reference)
"""BertSelfAttention (B=2, S=2048, HID=1024, NH=16, HD=64, SKV=2048) on 8 TRN2 NeuronCores.

Sharding: tensor-parallel over heads — 2 heads per core. Each core projects its
own 128 output channels of Q/K/V from the full hidden states, runs attention for
its 2 heads against the (sharded) kv cache + fresh K/V, and writes a [B, 128, S]
transposed context slice. The host concatenates the 8 slices along hidden dim.

On-device layout (per core):
  - qT/kT: [128 (2 heads x 64 dims), B*S] with head h on partitions h*64:(h+1)*64.
    Head 0 / head 1 matmuls use PE row-tiles (64,0)/(64,64 base) in parallel.
  - scores computed transposed: scoresT[kv, q] = kT_chunk.T-contract @ qT,
    softmax denominators via an all-ones column appended to V (M=65 ctx matmul).
  - all matmul operands use float32r (full-rate fp32-reduced mode, err ~1e-5).
"""

import sys

sys.path.insert(0, "/opt/trn_rl_repo")

import numpy as np

B, S, HID, NH, HD, SKV = 2, 2048, 1024, 16, 64, 2048
NCORES = 8
P = 128
SC = 512                    # q-chunk width (fp32 moving operand max)
NSC = B * S // SC           # 8 column chunks of hsT
KO = HID // P               # 8 contraction chunks for projections
NJ = (SKV + S) // P         # 32 kv chunks per (b, h); 0..15 cache, 16..31 new
VJ = SKV // P               # 16 chunks per segment
NM = S // SC                # 4 q-chunks per batch
EXP_GROUPS = [3] * 10 + [2]  # kv-chunk grouping for exp ops (3*10+2 == NJ)

_prog_cache = {}


def _build_program():
    import concourse.bacc as bacc
    import concourse.mybir as mybir
    import concourse.tile as tile
    from concourse.masks import make_identity

    f32 = mybir.dt.float32
    f32r = mybir.dt.float32r
    Exp = mybir.ActivationFunctionType.Exp

    nc = bacc.Bacc("TRN2", target_bir_lowering=False, debug=False, num_devices=NCORES)

    hsT = nc.dram_tensor("hsT", [HID, B * S], f32r, kind="ExternalInput").ap()
    wq = nc.dram_tensor("wq", [HID, P], f32r, kind="ExternalInput").ap()
    wk = nc.dram_tensor("wk", [HID, P], f32r, kind="ExternalInput").ap()
    wv = nc.dram_tensor("wv", [HID, P], f32r, kind="ExternalInput").ap()
    bq = nc.dram_tensor("bq", [P, 1], f32, kind="ExternalInput").ap()
    bk = nc.dram_tensor("bk", [P, 1], f32, kind="ExternalInput").ap()
    bv = nc.dram_tensor("bv", [P, 1], f32, kind="ExternalInput").ap()
    onesp = nc.dram_tensor("onesp", [P, 1], f32r, kind="ExternalInput").ap()
    ktc = nc.dram_tensor("ktc", [P, B, SKV], f32r, kind="ExternalInput").ap()
    vc = nc.dram_tensor("vc", [B, SKV, 130], f32r, kind="ExternalInput").ap()
    out = nc.dram_tensor("out", [B, P, S], f32, kind="ExternalOutput").ap()

    with tile.TileContext(nc) as tc:
        with tc.tile_pool(name="persist", bufs=1) as persist:
            wq_sb = persist.tile([P, KO, P], f32r, tag="wq")
            wk_sb = persist.tile([P, KO, P], f32r, tag="wk")
            wv_sb = persist.tile([P, KO, P], f32r, tag="wv")
            nc.sync.dma_start(wq_sb[:], wq.rearrange("(ko p) m -> p ko m", p=P))
            nc.sync.dma_start(wk_sb[:], wk.rearrange("(ko p) m -> p ko m", p=P))
            nc.sync.dma_start(wv_sb[:], wv.rearrange("(ko p) m -> p ko m", p=P))
            bq_sb = persist.tile([P, 1], f32, tag="bq")
            bk_sb = persist.tile([P, 1], f32, tag="bk")
            bv_sb = persist.tile([P, 1], f32, tag="bv")
            nc.sync.dma_start(bq_sb[:], bq)
            nc.sync.dma_start(bk_sb[:], bk)
            nc.sync.dma_start(bv_sb[:], bv)
            ktc_sb = persist.tile([P, B, SKV], f32r, tag="ktc")
            # v layout: [p, b, seg, jo, 130]; cols 0:64 head0, 64 ones, 65:129 head1, 129 ones
            v_sb = persist.tile([P, B, 2, VJ, 130], f32r, tag="v")
            ones_sb = persist.tile([P, 1], f32r, tag="ones")

            qT_sb = persist.tile([P, NSC, SC], f32r, tag="qT")
            kTn_sb = persist.tile([P, NSC, SC], f32r, tag="kTn")
            identity = persist.tile([P, P], f32, tag="ident")
            make_identity(nc, identity[:])

            # ---------------- phase 1: QKV projections ----------------
            hsT_r = hsT.rearrange("(ko p) n -> p ko n", p=P)
            qT_w = qT_sb[:].rearrange("p a b -> p (a b)")
            kTn_w = kTn_sb[:].rearrange("p a b -> p (a b)")
            # first chunks narrowed so the first matmuls start sooner
            chunks = [(0, 256), (256, 256)] + [(i * SC, SC) for i in range(1, NSC)]
            with (
                tc.tile_pool(name="hst", bufs=3) as hpool,
                tc.tile_pool(name="p1ps", bufs=3, space="PSUM") as p1ps,
                tc.tile_pool(name="vt", bufs=2) as vtp,
                tc.tile_pool(name="tps", bufs=2, space="PSUM") as tpool,
            ):
                for off, cw in chunks:
                    hst = hpool.tile([P, KO, SC], f32r, tag="hst")
                    nc.sync.dma_start(hst[:, :, :cw], hsT_r[:, :, off:off + cw])
                    for w_sb, b_sb, dest in (
                        (wq_sb, bq_sb, qT_w),
                        (wk_sb, bk_sb, kTn_w),
                    ):
                        ps = p1ps.tile([P, SC], f32, tag="proj")
                        for ko in range(KO):
                            nc.tensor.matmul(
                                ps[:, :cw], w_sb[:, ko], hst[:, ko, :cw],
                                start=(ko == 0), stop=(ko == KO - 1),
                            )
                        nc.vector.tensor_add(
                            dest[:, off:off + cw], ps[:, :cw],
                            b_sb[:].to_broadcast((P, cw)),
                        )
                    # V: project transposed, then PE-transpose into row layout
                    ps = p1ps.tile([P, SC], f32, tag="proj")
                    for ko in range(KO):
                        nc.tensor.matmul(
                            ps[:, :cw], wv_sb[:, ko], hst[:, ko, :cw],
                            start=(ko == 0), stop=(ko == KO - 1),
                        )
                    vt = vtp.tile([P, SC], f32, tag="vt")
                    nc.vector.tensor_add(
                        vt[:, :cw], ps[:, :cw], bv_sb[:].to_broadcast((P, cw))
                    )
                    for t in range(cw // P):
                        tp = tpool.tile([P, P], f32, tag="tp")
                        nc.tensor.transpose(tp[:], vt[:, t * P:(t + 1) * P], identity[:])
                        base = off + t * P
                        b_i, jo = base // S, (base % S) // P
                        nc.vector.tensor_copy(out=v_sb[:, b_i, 1, jo, 0:64], in_=tp[:, 0:64])
                        nc.vector.tensor_copy(out=v_sb[:, b_i, 1, jo, 65:129], in_=tp[:, 64:128])

            # cache loads: emitted after phase 1 so they don't delay the first
            # hsT chunk on the DMA queues (only needed once phase 2 starts)
            nc.sync.dma_start(ktc_sb[:], ktc)
            for b_i in range(B):
                nc.sync.dma_start(
                    v_sb[:, b_i, 0], vc[b_i].rearrange("(jo p) c -> p jo c", p=P)
                )
            nc.sync.dma_start(ones_sb[:], onesp)
            nc.vector.tensor_copy(
                out=v_sb[:, :, 1, :, 64:65],
                in_=ones_sb[:, :, None, None].to_broadcast((P, B, VJ, 1)),
            )
            nc.vector.tensor_copy(
                out=v_sb[:, :, 1, :, 129:130],
                in_=ones_sb[:, :, None, None].to_broadcast((P, B, VJ, 1)),
            )

            # ---------------- phase 2: attention ----------------
            qT_f = qT_sb[:].rearrange("p a b -> p (a b)")
            kTn_f = kTn_sb[:].rearrange("p a b -> p (a b)")
            with (
                tc.tile_pool(name="scps", bufs=1, space="PSUM") as scps,
                tc.tile_pool(name="ctxps", bufs=1, space="PSUM") as ctxps,
                tc.tile_pool(name="probs", bufs=3) as probp,
                tc.tile_pool(name="norm", bufs=2) as normp,
            ):
                for b in range(B):
                    for m in range(NM):
                        q0 = b * S + m * SC
                        ctx = [
                            ctxps.tile([P, SC], f32, tag=f"ctx{h}", name=f"ctx{h}")
                            for h in range(2)
                        ]

                        def emit_ctx(h, j0, g, pr):
                            for jj in range(g):
                                jg = j0 + jj
                                seg, jo = (0, jg) if jg < VJ else (1, jg - VJ)
                                nc.tensor.matmul(
                                    ctx[h][0:65, :],
                                    v_sb[:, b, seg, jo, h * 65:(h + 1) * 65],
                                    pr[:, jj],
                                    start=(jg == 0), stop=(jg == NJ - 1),
                                )

                        j = 0
                        pending = []
                        for g in EXP_GROUPS:
                            nxt = []
                            for h in range(2):
                                hs0, hs1 = h * 64, (h + 1) * 64
                                sct = scps.tile([P, 3, SC], f32, tag=f"sc{h}")
                                for jj in range(g):
                                    jg = j + jj
                                    if jg < VJ:
                                        lhsT = ktc_sb[hs0:hs1, b, jg * P:(jg + 1) * P]
                                    else:
                                        col = b * S + (jg - VJ) * P
                                        lhsT = kTn_f[hs0:hs1, col:col + P]
                                    nc.tensor.matmul(
                                        sct[:, jj], lhsT, qT_f[hs0:hs1, q0:q0 + SC],
                                        start=True, stop=True,
                                    )
                                pr = probp.tile([P, 3, SC], f32r, tag=f"pr{h}")
                                nc.scalar.activation(
                                    pr[:, :g], sct[:, :g], Exp, scale=0.125
                                )
                                nxt.append((h, j, g, pr))
                            # ctx for the previous group — keeps PE a group ahead
                            for args in pending:
                                emit_ctx(*args)
                            pending = nxt
                            j += g
                        for args in pending:
                            emit_ctx(*args)
                        for h in range(2):
                            # one quick copy releases the ctx PSUM bank early
                            tmp = normp.tile([65, SC], f32, tag="tmp")
                            nc.vector.tensor_copy(out=tmp[:], in_=ctx[h][0:65, :])
                            recip = normp.tile([1, SC], f32, tag="recip")
                            nc.vector.reciprocal(recip[:], tmp[64:65, :])
                            rbc = normp.tile([64, SC], f32, tag="rbc")
                            nc.gpsimd.partition_broadcast(rbc[:], recip[:])
                            res = normp.tile([64, SC], f32, tag="res")
                            nc.vector.tensor_mul(res[:], tmp[0:64, :], rbc[:])
                            nc.sync.dma_start(
                                out[b, h * 64:(h + 1) * 64, m * SC:(m + 1) * SC], res[:]
                            )

    nc.compile()
    return nc


def get_program():
    if "nc" not in _prog_cache:
        _prog_cache["nc"] = _build_program()
    return _prog_cache["nc"]


def make_in_maps(hidden_states, kvs, Wq, bq, Wk, bk, Wv, bv, kv_weight):
    hs = np.asarray(hidden_states, np.float32).reshape(B * S, HID)
    hsT = np.ascontiguousarray(hs.T)
    kvw = np.float32(kv_weight)
    Wq = np.asarray(Wq, np.float32)
    Wk = np.asarray(Wk, np.float32)
    Wv = np.asarray(Wv, np.float32)
    bq = np.asarray(bq, np.float32)
    bk = np.asarray(bk, np.float32)
    bv = np.asarray(bv, np.float32)
    kvs = np.asarray(kvs, np.float32)
    scale = np.float32(HD ** -0.5)

    in_maps = []
    for c in range(NCORES):
        rows = slice(c * P, (c + 1) * P)
        h0, h1 = 2 * c, 2 * c + 1
        wq_c = np.ascontiguousarray((Wq[rows] * scale).T)       # [HID, 128]
        wk_c = np.ascontiguousarray(Wk[rows].T)
        wv_c = np.ascontiguousarray(Wv[rows].T)
        bq_c = np.ascontiguousarray((bq[rows] * scale).reshape(P, 1))
        bk_c = np.ascontiguousarray(bk[rows].reshape(P, 1))
        bv_c = np.ascontiguousarray(bv[rows].reshape(P, 1))
        # k cache transposed: [128 (h,d), B, SKV]
        kc = kvs[0][:, [h0, h1]] * kvw                           # [B, 2, SKV, HD]
        ktc_c = np.ascontiguousarray(kc.transpose(1, 3, 0, 2).reshape(P, B, SKV))
        # v cache with ones columns: [B, SKV, 130]
        vcache = kvs[1][:, [h0, h1]] * kvw                       # [B, 2, SKV, HD]
        vc_c = np.empty((B, SKV, 130), np.float32)
        vc_c[:, :, 0:64] = vcache[:, 0]
        vc_c[:, :, 64] = 1.0
        vc_c[:, :, 65:129] = vcache[:, 1]
        vc_c[:, :, 129] = 1.0
        in_maps.append({
            "hsT": hsT, "wq": wq_c, "wk": wk_c, "wv": wv_c,
            "bq": bq_c, "bk": bk_c, "bv": bv_c,
            "onesp": np.ones((P, 1), np.float32),
            "ktc": ktc_c, "vc": vc_c,
        })
    return in_maps


def assemble_output(results):
    full = np.empty((B, S, HID), np.float32)
    for c in range(NCORES):
        o = results[c]["out"]                                    # [B, 128, S]
        full[:, :, c * P:(c + 1) * P] = o.transpose(0, 2, 1)
    return full


def kernel(hidden_states, kvs, Wq, bq, Wk, bk, Wv, bv, kv_weight, _trace=False):
    from concourse.bass_utils import run_bass_kernel_spmd

    nc = get_program()
    in_maps = make_in_maps(hidden_states, kvs, Wq, bq, Wk, bk, Wv, bv, kv_weight)
    res = run_bass_kernel_spmd(nc, in_maps, list(range(NCORES)), trace=_trace)
    outp = assemble_output(res.results)
    if _trace:
        kernel.last_results = res
    return outp
